# revision 23
# baseline (speedup 1.0000x reference)
"""ConditionalLM decode kernel for 8 Trainium2 NeuronCores.

Strategy (v4):
  - Vocab-shard W_pred across 8 cores (4096 cols each, zero-padded); the
    prediction matmul runs in float32r (1 cyc/row vs 4 for fp32); per-block
    top-8 scans ride under the matmul, and the top-2 local candidates are
    re-scored with an exact fp32 dot product so fp32r rounding (~3.5e-7
    logit noise) cannot flip the argmax (top-2 margins go down to 5e-8).
  - GRU sharded 8 ways by gate feature columns (64 h-features per core,
    permuted weight columns r_c|z_c|n_c per core), row-major exact fp32;
    h' slices exchanged with a 32KB AllGather per stream-step.  h_old is
    selected with a one-hot matrix appended to the whh upload (SPMD
    programs cannot use per-core addresses).  PSUM zero regions are 2KB:
    exactly one start=True per bank per step.
  - Batch split into 2 streams of 128 rows; emission is interleaved per
    sub-phase so engine FIFOs never head-of-line block the other stream.
  - Cross-core argmax: (exact val, global idx) pairs, AllGather + local
    combine; ties resolve to the smallest vocab index, matching jnp.argmax.
"""
import numpy as np

VOCAB = 32002
H = 512
COND = 1024
MAXLEN = 15
B = 256
NCORES = 8
NSHARD = 4096          # per-core vocab shard (8*4096 = 32768 >= 32002)
NSTEPS = MAXLEN - 1    # 14 decode steps
P = 128
F = 64                 # h-features per core (512/8)
STREAMS = (0, 1)
KT = 4                 # hidden k-tiles (512/128)
KC = 8                 # cond k-tiles (1024/128)
NT = NSHARD // 512     # 8 pred n-tiles


def _build(bcond_nz=False, bgate_nz=False, bpred_nz=False):
    import concourse.bacc as bacc
    import concourse.mybir as mybir
    from concourse.tile import TileContext
    from concourse.bass import IndirectOffsetOnAxis

    f32 = mybir.dt.float32
    f32r = mybir.dt.float32r
    i32 = mybir.dt.int32
    u32 = mybir.dt.uint32
    AF = mybir.ActivationFunctionType
    OP = mybir.AluOpType
    AxisX = mybir.AxisListType.X

    nc = bacc.Bacc("TRN2", target_bir_lowering=False, debug=True, num_devices=NCORES)

    # ---------------- I/O ----------------
    emb = nc.declare_dram_parameter("emb", [VOCAB, H], f32, isOutput=False)
    wpt = nc.declare_dram_parameter("wpt", [H, NSHARD], f32, isOutput=False)
    wrows = nc.declare_dram_parameter("wrows", [NSHARD, H + 1], f32, isOutput=False)
    wih_r = nc.declare_dram_parameter("wih_r", [H, 3 * F], f32, isOutput=False)
    whh_r = nc.declare_dram_parameter("whh_r", [H, 3 * F + F], f32, isOutput=False)
    wct = nc.declare_dram_parameter("wct", [COND, H], f32, isOutput=False)
    imgT_d = nc.declare_dram_parameter("imgT", [COND, B], f32, isOutput=False)
    tok0 = nc.declare_dram_parameter("tok0", [B], i32, isOutput=False)
    base_t = nc.declare_dram_parameter("base_t", [P, 1], i32, isOutput=False)
    ident_in = nc.declare_dram_parameter("ident_in", [P, P], f32, isOutput=False)
    if bcond_nz:
        bcond_row = nc.declare_dram_parameter("bcond_row", [1, H], f32, isOutput=False)
    if bgate_nz:
        bgate_row = nc.declare_dram_parameter("bgate_row", [1, 4 * F], f32, isOutput=False)
    if bpred_nz:
        bpred_row = nc.declare_dram_parameter("bpred_row", [1, NSHARD], f32, isOutput=False)
    preds = nc.declare_dram_parameter("preds", [B, MAXLEN], i32, isOutput=True)

    # internal DRAM for collectives (one pair per stream-step, static)
    k_in = [[nc.dram_tensor(f"k_in_{t}_{s}", [P * 2], f32) for s in STREAMS]
            for t in range(NSTEPS)]
    k_out = [[nc.dram_tensor(f"k_out_{t}_{s}", [NCORES * P * 2], f32,
                             addr_space="Shared") for s in STREAMS]
             for t in range(NSTEPS)]
    h_in = [[nc.dram_tensor(f"h_in_{t}_{s}", [2 * P * F], f32) for s in STREAMS]
            for t in range(NSTEPS)]
    h_out = [[nc.dram_tensor(f"h_out_{t}_{s}", [NCORES * 2 * P * F], f32,
                             addr_space="Shared") for s in STREAMS]
             for t in range(NSTEPS)]

    with TileContext(nc) as tc:
        with (
            tc.tile_pool(name="wts", bufs=1) as wts,
            tc.tile_pool(name="work", bufs=1) as work,
            tc.tile_pool(name="sc", bufs=1) as sc,
            tc.tile_pool(name="psg", bufs=1, space="PSUM") as psg,
            tc.tile_pool(name="pst", bufs=1, space="PSUM") as pst,
            tc.tile_pool(name="psr", bufs=4, space="PSUM") as psr,
        ):
            # ================= resident weights =================
            base_sb = wts.tile([P, 1], i32, tag="base", name="base")
            nc.sync.dma_start(out=base_sb[:], in_=base_t[:])
            ident = wts.tile([P, P], f32, tag="ident", name="ident")
            nc.sync.dma_start(out=ident[:], in_=ident_in[:])

            wih_sb = wts.tile([P, KT * 3 * F], f32, tag="wih", name="wih")
            whh_sb = wts.tile([P, KT * 4 * F], f32, tag="whh", name="whh")
            for k in range(KT):
                nc.sync.dma_start(out=wih_sb[:, k * 3 * F:(k + 1) * 3 * F],
                                  in_=wih_r[k * P:(k + 1) * P, :])
                nc.sync.dma_start(out=whh_sb[:, k * 4 * F:(k + 1) * 4 * F],
                                  in_=whh_r[k * P:(k + 1) * P, :])

            # prediction weights: stage fp32, cast to f32r
            wpt_r = [wts.tile([P, NSHARD], f32r, tag=f"wptr{k}", name=f"wptr{k}")
                     for k in range(KT)]
            with tc.tile_pool(name="stage", bufs=2) as stage:
                for k in range(KT):
                    st = stage.tile([P, NSHARD], f32, tag="st", name="st")
                    nc.sync.dma_start(out=st[:], in_=wpt[k * P:(k + 1) * P, :])
                    nc.vector.tensor_copy(wpt_r[k][:], st[:])

            if bcond_nz or bgate_nz or bpred_nz:
                ones_col = wts.tile([1, P], f32, tag="ones", name="ones")
                nc.vector.memset(ones_col[:], 1.0)
            if bcond_nz:
                bcr_sb = wts.tile([1, H], f32, tag="bcr", name="bcr")
                nc.sync.dma_start(out=bcr_sb[:], in_=bcond_row[:])
            if bgate_nz:
                bgr_sb = wts.tile([1, 4 * F], f32, tag="bgr", name="bgr")
                nc.sync.dma_start(out=bgr_sb[:], in_=bgate_row[:])
            if bpred_nz:
                ones_col_r = wts.tile([1, P], f32r, tag="onesr", name="onesr")
                nc.vector.memset(ones_col_r[:], 1.0)
                bpr_sb = wts.tile([1, NSHARD], f32r, tag="bpr", name="bpr")
                st2 = wts.tile([1, NSHARD], f32, tag="bprs", name="bprs")
                nc.sync.dma_start(out=st2[:], in_=bpred_row[:])
                nc.vector.tensor_copy(bpr_sb[:], st2[:])

            tok_sb = [work.tile([P, 1], i32, tag=f"tok{s}", name=f"tok{s}")
                      for s in STREAMS]
            tokh = [work.tile([P, MAXLEN], i32, tag=f"tokh{s}", name=f"tokh{s}")
                    for s in STREAMS]
            for s in STREAMS:
                nc.sync.dma_start(out=tok_sb[s][:], in_=tok0[s * P:(s + 1) * P][:, None])
                nc.vector.tensor_copy(tokh[s][:, 0:1], tok_sb[s][:])

            # persistent state (h_row col H holds 1.0 for the bias-augmented
            # rescore dot)
            h_row = [work.tile([P, H + 1], f32, tag=f"hrow{s}", name=f"hrow{s}")
                     for s in STREAMS]
            hT = [work.tile([P, H], f32, tag=f"hT{s}", name=f"hT{s}")
                  for s in STREAMS]
            hTr = [work.tile([P, H], f32r, tag=f"hTr{s}", name=f"hTr{s}")
                   for s in STREAMS]
            xT = [work.tile([P, H], f32, tag=f"xT{s}", name=f"xT{s}")
                  for s in STREAMS]
            for s in STREAMS:
                nc.vector.memset(h_row[s][:, H:H + 1], 1.0)

            def transpose_to_hT(src_row, s):
                ps_tp = psr.tile([P, H], f32, tag="pred", name=f"tp{s}")
                for j in range(KT):
                    nc.tensor.transpose(ps_tp[:, j * P:(j + 1) * P],
                                        src_row[:, j * P:(j + 1) * P], ident[:])
                nc.scalar.activation(hT[s][:], ps_tp[:], AF.Copy)
                nc.vector.tensor_copy(hTr[s][:], ps_tp[:])

            # ================= h0 = img @ W_cond.T (row-major) =================
            with tc.tile_pool(name="setup", bufs=1) as setup:
                wct_sb = [setup.tile([P, H], f32, tag=f"wct{k}", name=f"wct{k}")
                          for k in range(KC)]
                imgT_sb = [setup.tile([P, B], f32, tag=f"img{k}", name=f"img{k}")
                           for k in range(KC)]
                for k in range(KC):
                    nc.sync.dma_start(out=wct_sb[k][:], in_=wct[k * P:(k + 1) * P, :])
                    nc.sync.dma_start(out=imgT_sb[k][:], in_=imgT_d[k * P:(k + 1) * P, :])
                for s in STREAMS:
                    ps_h0 = psr.tile([P, H], f32, tag="pred", name="ps_h0")
                    for k in range(KC):
                        nc.tensor.matmul(
                            ps_h0[:], lhsT=imgT_sb[k][:, s * P:(s + 1) * P],
                            rhs=wct_sb[k][:], start=(k == 0),
                            stop=(k == KC - 1 and not bcond_nz))
                    if bcond_nz:
                        nc.tensor.matmul(ps_h0[:], lhsT=ones_col[:],
                                         rhs=bcr_sb[:], start=False, stop=True)
                    nc.scalar.activation(h_row[s][:, 0:H], ps_h0[:], AF.Copy)
                    transpose_to_hT(h_row[s][:, 0:H], s)

            # ================= decode steps =================
            ps_g = [None, None]

            def emit_gh(t, s):
                ps_g[s] = psg.tile([P, 5 * F + P], f32, tag=f"g{s}", name=f"g{s}")
                pg = ps_g[s]
                for k in range(KT):
                    nc.tensor.matmul(
                        pg[:, 0:2 * F], lhsT=hT[s][:, k * P:(k + 1) * P],
                        rhs=whh_sb[:, k * 4 * F:k * 4 * F + 2 * F],
                        start=(k == 0), stop=False)
                    nc.tensor.matmul(
                        pg[:, 3 * F:5 * F], lhsT=hT[s][:, k * P:(k + 1) * P],
                        rhs=whh_sb[:, k * 4 * F + 2 * F:(k + 1) * 4 * F],
                        start=False, stop=False)

            def emit_gather_x(t, s):
                x_sb = sc.tile([P, H], f32, tag=f"x{s}", name=f"x{s}")
                nc.gpsimd.indirect_dma_start(
                    out=x_sb[:], out_offset=None, in_=emb[:],
                    in_offset=IndirectOffsetOnAxis(ap=tok_sb[s][:, :1], axis=0))
                ps_tp = pst.tile([P, H], f32, tag=f"xp{s}", name=f"tpx{s}")
                for j in range(KT):
                    nc.tensor.transpose(ps_tp[:, j * P:(j + 1) * P],
                                        x_sb[:, j * P:(j + 1) * P], ident[:])
                nc.scalar.activation(xT[s][:], ps_tp[:], AF.Copy)

            def emit_gi(t, s):
                pg = ps_g[s]
                for k in range(KT):
                    nc.tensor.matmul(
                        pg[:, 0:2 * F], lhsT=xT[s][:, k * P:(k + 1) * P],
                        rhs=wih_sb[:, k * 3 * F:k * 3 * F + 2 * F],
                        start=False, stop=False)
                    nc.tensor.matmul(
                        pg[:, 2 * F:3 * F], lhsT=xT[s][:, k * P:(k + 1) * P],
                        rhs=wih_sb[:, k * 3 * F + 2 * F:(k + 1) * 3 * F],
                        start=False, stop=(k == KT - 1 and not bgate_nz))
                if bgate_nz:
                    nc.tensor.matmul(pg[:, 0:2 * F], lhsT=ones_col[:],
                                     rhs=bgr_sb[:, 0:2 * F], start=False, stop=False)
                    nc.tensor.matmul(pg[:, 2 * F:3 * F], lhsT=ones_col[:],
                                     rhs=bgr_sb[:, 2 * F:3 * F], start=False, stop=False)
                    nc.tensor.matmul(pg[:, 3 * F:4 * F], lhsT=ones_col[:],
                                     rhs=bgr_sb[:, 3 * F:4 * F], start=False, stop=True)

            def emit_gates(t, s):
                # no vector-engine ops here: ph1 must stay off the vector FIFO
                pg = ps_g[s]
                rz_sb = sc.tile([P, 2 * F], f32, tag=f"rz{s}", name=f"rz{s}")
                nc.scalar.activation(rz_sb[:], pg[:, 0:2 * F], AF.Sigmoid)
                ih_sb = sc.tile([P, 3 * F], f32, tag=f"ih{s}", name=f"ih{s}")
                nc.scalar.activation(ih_sb[:], pg[:, 2 * F:5 * F], AF.Copy)
                t2_sb = sc.tile([P, F], f32, tag=f"t2{s}", name=f"t2{s}")
                nc.gpsimd.tensor_mul(t2_sb[:], rz_sb[:, 0:F], ih_sb[:, F:2 * F])
                nc.gpsimd.tensor_add(t2_sb[:], t2_sb[:], ih_sb[:, 0:F])
                n_sb = sc.tile([P, F], f32, tag=f"n{s}", name=f"n{s}")
                nc.scalar.activation(n_sb[:], t2_sb[:], AF.Tanh)
                # h' = n + z*(h_old - n)
                d_sb = sc.tile([P, F], f32, tag=f"d{s}", name=f"d{s}")
                nc.gpsimd.tensor_sub(d_sb[:], ih_sb[:, 2 * F:3 * F], n_sb[:])
                nc.gpsimd.tensor_mul(d_sb[:], d_sb[:], rz_sb[:, F:2 * F])
                nc.gpsimd.tensor_add(d_sb[:], d_sb[:], n_sb[:])
                # transposed copy of the slice so hT can be assembled by DMA
                # (parked in the spare columns of the gates psum bank)
                nc.tensor.transpose(pg[0:F, 5 * F:5 * F + P], d_sb[:], ident[:])
                dT_sb = sc.tile([F, P], f32, tag=f"dT{s}", name=f"dT{s}")
                nc.scalar.activation(dT_sb[:], pg[0:F, 5 * F:5 * F + P], AF.Copy)
                return d_sb, dT_sb

            def emit_hshare(t, s, d_pair):
                d_sb, dT_sb = d_pair
                nc.sync.dma_start(
                    out=h_in[t][s][0:P * F].rearrange("(p f) -> p f", f=F),
                    in_=d_sb[:])
                nc.sync.dma_start(
                    out=h_in[t][s][P * F:2 * P * F].rearrange("(f p) -> f p", p=P),
                    in_=dT_sb[:])
                nc.gpsimd.collective_compute(
                    "AllGather", OP.bypass,
                    replica_groups=[list(range(NCORES))],
                    ins=[h_in[t][s][:]], outs=[h_out[t][s][:]])

            def emit_hback(t, s):
                hv = h_out[t][s][:].rearrange("(c r) -> c r", r=2 * P * F)
                # row-major part (per-core [p, f] blocks)
                nc.sync.dma_start(
                    out=h_row[s][:, 0:H].rearrange("p (c f) -> p c f", f=F),
                    in_=hv[:, 0:P * F].rearrange("c (p f) -> p c f", f=F))
                # transposed part (per-core [f, p] blocks) -> hT directly,
                # one DMA per partition half (partition dim cannot split)
                tv = hv[:, P * F:2 * P * F].rearrange(
                    "(c2 cl) (f p) -> cl f c2 p", cl=2, p=P)
                for cl in range(2):
                    nc.sync.dma_start(
                        out=hT[s][cl * F:(cl + 1) * F, :].rearrange(
                            "f (c2 p) -> f c2 p", c2=4),
                        in_=tv[cl])
                nc.scalar.activation(hTr[s][:], hT[s][:], AF.Copy)

            def emit_pred(t, s):
                """fp32r pred with per-block top-8 scans riding under it."""
                bt2 = sc.tile([P, 2 * NT], f32, tag=f"bt2{s}", name=f"bt2{s}")
                bi2 = sc.tile([P, 2 * NT], i32, tag=f"bi2{s}", name=f"bi2{s}")
                for n in range(NT):
                    ps_pred = psr.tile([P, 512], f32, tag="pred", name="pred")
                    for k in range(KT):
                        nc.tensor.matmul(
                            ps_pred[:], lhsT=hTr[s][:, k * P:(k + 1) * P],
                            rhs=wpt_r[k][:, n * 512:(n + 1) * 512],
                            start=(k == 0),
                            stop=(k == KT - 1 and not bpred_nz))
                    if bpred_nz:
                        nc.tensor.matmul(
                            ps_pred[:], lhsT=ones_col_r[:],
                            rhs=bpr_sb[:, n * 512:(n + 1) * 512],
                            start=False, stop=True)
                    bm = sc.tile([P, 8], f32, tag=f"bm{s}_{n % 2}", name=f"bm{s}_{n}")
                    bi = sc.tile([P, 8], u32, tag=f"bi{s}_{n % 2}", name=f"bi{s}_{n}")
                    nc.vector.max(out=bm[:], in_=ps_pred[:])
                    nc.vector.max_index(out=bi[:], in_max=bm[:], in_values=ps_pred[:])
                    nc.vector.tensor_copy(bt2[:, 2 * n:2 * n + 2], bm[:, 0:2])
                    nc.vector.tensor_scalar_add(bi2[:, 2 * n:2 * n + 2],
                                                bi[:, 0:2].bitcast(i32), n * 512)
                return bt2, bi2

            def emit_argmax_rescore(t, s, bt2, bi2):
                # local top-2 across the 8 blocks; launch gathers eagerly
                g8 = sc.tile([P, 8], f32, tag=f"g8{s}", name=f"g8{s}")
                nc.vector.max(out=g8[:], in_=bt2[:])
                li = sc.tile([P, 2], i32, tag=f"li{s}", name=f"li{s}")
                w1 = sc.tile([P, H + 1], f32, tag=f"w1{s}", name=f"w1{s}")
                w2 = sc.tile([P, H + 1], f32, tag=f"w2{s}", name=f"w2{s}")
                for j, wj in ((0, w1), (1, w2)):
                    msk = sc.tile([P, 2 * NT], u32, tag=f"msk{s}", name=f"msk{s}")
                    nc.vector.tensor_tensor(msk[:], bt2[:],
                                            g8[:, j:j + 1].to_broadcast([P, 2 * NT]),
                                            OP.is_ge)
                    cnd = sc.tile([P, 2 * NT], i32, tag=f"cnd{s}", name=f"cnd{s}")
                    nc.vector.memset(cnd[:], 0x7FFFFFFF)
                    nc.vector.copy_predicated(cnd[:], msk[:], bi2[:])
                    nc.vector.tensor_reduce(li[:, j:j + 1], cnd[:], AxisX, OP.min)
                    nc.gpsimd.indirect_dma_start(
                        out=wj[:], out_offset=None, in_=wrows[:],
                        in_offset=IndirectOffsetOnAxis(ap=li[:, j:j + 1], axis=0))
                nc.vector.tensor_mul(w1[:], w1[:], h_row[s][:])
                nc.vector.tensor_mul(w2[:], w2[:], h_row[s][:])
                ev = sc.tile([P, 2], f32, tag=f"ev{s}", name=f"ev{s}")
                nc.vector.tensor_reduce(ev[:, 0:1], w1[:], AxisX, OP.add)
                nc.vector.tensor_reduce(ev[:, 1:2], w2[:], AxisX, OP.add)
                # key = (max exact val, its global idx; tie -> min idx)
                gi2 = sc.tile([P, 2], i32, tag=f"gi2{s}", name=f"gi2{s}")
                nc.vector.tensor_add(gi2[:, 0:1], li[:, 0:1], base_sb[:])
                nc.vector.tensor_add(gi2[:, 1:2], li[:, 1:2], base_sb[:])
                key = sc.tile([P, 2], f32, tag=f"key{s}", name=f"key{s}")
                nc.vector.tensor_reduce(key[:, 0:1], ev[:], AxisX, OP.max)
                mk2 = sc.tile([P, 2], u32, tag=f"mk2{s}", name=f"mk2{s}")
                nc.vector.tensor_tensor(mk2[:], ev[:],
                                        key[:, 0:1].to_broadcast([P, 2]), OP.is_ge)
                cn2 = sc.tile([P, 2], i32, tag=f"cn2{s}", name=f"cn2{s}")
                nc.vector.memset(cn2[:], 0x7FFFFFFF)
                nc.vector.copy_predicated(cn2[:], mk2[:], gi2[:])
                nc.vector.tensor_reduce(key[:, 1:2].bitcast(i32), cn2[:], AxisX, OP.min)
                nc.sync.dma_start(
                    out=k_in[t][s][:].rearrange("(p w) -> p w", w=2),
                    in_=key[:])
                nc.gpsimd.collective_compute(
                    "AllGather", OP.bypass,
                    replica_groups=[list(range(NCORES))],
                    ins=[k_in[t][s][:]], outs=[k_out[t][s][:]])

            def emit_combine(t, s):
                # entirely on gpsimd: chains into the gpsimd x-gather with no
                # cross-engine hop, and stays off the congested vector FIFO
                kv = sc.tile([P, 2 * NCORES], f32, tag=f"kv{s}", name=f"kv{s}")
                nc.sync.dma_start(
                    out=kv[:].rearrange("p (c w) -> p c w", w=2),
                    in_=k_out[t][s][:].rearrange("(c p w) -> p c w", c=NCORES, w=2))
                vals8 = kv[:].rearrange("p (c w) -> p c w", w=2)[:, :, 0]
                idx8 = kv[:].rearrange("p (c w) -> p c w", w=2)[:, :, 1].bitcast(i32)
                gmax = sc.tile([P, 1], f32, tag=f"gm{s}", name=f"gm{s}")
                nc.vector.tensor_reduce(gmax[:], vals8, AxisX, OP.max)
                mask = sc.tile([P, NCORES], u32, tag=f"mk{s}", name=f"mk{s}")
                nc.vector.tensor_tensor(mask[:], vals8,
                                        gmax[:].to_broadcast([P, NCORES]), OP.is_ge)
                cand = sc.tile([P, NCORES], i32, tag=f"cd{s}", name=f"cd{s}")
                nc.vector.memset(cand[:], 0x7FFFFFFF)
                nc.vector.copy_predicated(cand[:], mask[:], idx8)
                tok_new = work.tile([P, 1], i32, tag=f"tok{s}", name=f"tok{s}")
                nc.vector.tensor_reduce(tok_new[:], cand[:], AxisX, OP.min)
                tok_sb[s] = tok_new
                nc.vector.tensor_copy(tokh[s][:, t + 1:t + 2], tok_new[:])

            def ph1(t, s, with_gh=True):
                """GRU half: tok(t) -> h'(t) -> coll2 launch.  No vector ops."""
                if with_gh:
                    emit_gh(t, s)
                emit_gather_x(t, s)
                emit_gi(t, s)
                d_pair = emit_gates(t, s)
                emit_hshare(t, s, d_pair)

            bt_c = [None, None]

            def ph2a(t, s):
                """h(t) back -> gh(t+1) hoisted -> logits + riding scans."""
                emit_hback(t, s)
                if t + 1 < NSTEPS:
                    emit_gh(t + 1, s)
                bt_c[s] = emit_pred(t, s)

            def ph2b(t, s):
                """top-2 select -> exact rescore -> key -> coll1 launch."""
                emit_argmax_rescore(t, s, *bt_c[s])

            # software-pipelined half-phase schedule, emitted in expected
            # execution order so no engine FIFO head-of-line blocks the
            # other stream
            ph1(0, 0)
            for t in range(NSTEPS):
                ph2a(t, 0)
                ph1(t, 1, with_gh=(t == 0))
                ph2b(t, 0)
                emit_combine(t, 0)
                ph2a(t, 1)
                if t + 1 < NSTEPS:
                    ph1(t + 1, 0, with_gh=False)
                ph2b(t, 1)
                emit_combine(t, 1)

            # final: write predictions once per stream
            for s in STREAMS:
                nc.sync.dma_start(out=preds[s * P:(s + 1) * P, :], in_=tokh[s][:])

    return nc


def _prep_inputs(caption, img, embedding, W_cond, b_cond, w_ih, w_hh, b_ih,
                 b_hh, W_pred, b_pred):
    caption = np.asarray(caption).astype(np.int32)
    img = np.ascontiguousarray(np.asarray(img, dtype=np.float32))
    embedding = np.ascontiguousarray(np.asarray(embedding, dtype=np.float32))
    W_pred = np.asarray(W_pred, dtype=np.float32)
    b_pred = np.asarray(b_pred, np.float32)
    b_ih = np.asarray(b_ih, np.float32)
    b_hh = np.asarray(b_hh, np.float32)
    wihT = np.ascontiguousarray(np.asarray(w_ih, np.float32).T)   # [H, 3H]
    whhT = np.ascontiguousarray(np.asarray(w_hh, np.float32).T)
    common = dict(
        emb=embedding,
        wct=np.ascontiguousarray(np.asarray(W_cond, np.float32).T),
        imgT=np.ascontiguousarray(img.T),
        tok0=np.ascontiguousarray(caption[:, 0]),
        ident_in=np.eye(P, dtype=np.float32),
        bcond_row=np.asarray(b_cond, np.float32).reshape(1, H),
    )
    in_maps = []
    for c in range(NCORES):
        base = c * NSHARD
        hi = min(base + NSHARD, VOCAB)
        n_real = max(0, hi - base)
        wpt_c = np.zeros((H, NSHARD), np.float32)
        wpt_c[:, :n_real] = W_pred[base:hi].T
        wrows_c = np.zeros((NSHARD, H + 1), np.float32)
        wrows_c[:n_real, :H] = W_pred[base:hi]
        wrows_c[:n_real, H] = b_pred[base:hi]
        lo, hi_f = c * F, (c + 1) * F
        wih_rc = np.concatenate(
            [wihT[:, lo:hi_f], wihT[:, H + lo:H + hi_f],
             wihT[:, 2 * H + lo:2 * H + hi_f]], axis=1)
        sel = np.zeros((H, F), np.float32)
        sel[lo:hi_f, :] = np.eye(F, dtype=np.float32)
        whh_rc = np.concatenate(
            [whhT[:, lo:hi_f], whhT[:, H + lo:H + hi_f],
             whhT[:, 2 * H + lo:2 * H + hi_f], sel], axis=1)
        bg = b_ih + b_hh
        bgate_row = np.concatenate(
            [bg[lo:hi_f], bg[H + lo:H + hi_f],
             b_ih[2 * H + lo:2 * H + hi_f], b_hh[2 * H + lo:2 * H + hi_f]])
        m = dict(common)
        m["wpt"] = np.ascontiguousarray(wpt_c)
        m["wrows"] = np.ascontiguousarray(wrows_c)
        m["wih_r"] = np.ascontiguousarray(wih_rc)
        m["whh_r"] = np.ascontiguousarray(whh_rc)
        m["base_t"] = np.full((P, 1), base, np.int32)
        m["bgate_row"] = np.ascontiguousarray(bgate_row.reshape(1, 4 * F))
        bp = np.zeros((1, NSHARD), np.float32)
        bp[0, :n_real] = b_pred[base:hi]
        m["bpred_row"] = bp
        in_maps.append(m)
    return in_maps


_CACHED = {}


def kernel(**inputs) -> np.ndarray:
    from concourse.bass_utils import run_bass_kernel_spmd

    in_maps = _prep_inputs(**inputs)
    bcond_nz = bool(np.any(np.asarray(inputs["b_cond"])))
    bgate_nz = bool(np.any(np.asarray(inputs["b_ih"]))
                    or np.any(np.asarray(inputs["b_hh"])))
    bpred_nz = bool(np.any(np.asarray(inputs["b_pred"])))
    key = (bcond_nz, bgate_nz, bpred_nz)
    if key not in _CACHED:
        nc = _build(*key)
        nc.finalize()
        _CACHED[key] = nc
    flags = ("bcond_row", "bgate_row", "bpred_row")
    drop = [f for f, nz in zip(flags, key) if not nz]
    for m in in_maps:
        for f in drop:
            m.pop(f, None)
    res = run_bass_kernel_spmd(_CACHED[key], in_maps, list(range(NCORES)))
    return np.ascontiguousarray(res.results[0]["preds"].astype(np.int32))


if __name__ == "__main__":
    d = np.load("inputs.npz")
    inputs = {k: d[k] for k in d.files}
    out = kernel(**inputs)
    exp = np.load("expected.npy")
    print("match:", np.array_equal(out, exp),
          " mismatches:", int((out != exp).sum()), "/", out.size)


# revision 24
# speedup vs baseline: 1.0188x; 1.0188x over previous
"""ConditionalLM decode kernel for 8 Trainium2 NeuronCores.

Strategy (v4):
  - Vocab-shard W_pred across 8 cores (4096 cols each, zero-padded); the
    prediction matmul runs in float32r (1 cyc/row vs 4 for fp32); per-block
    top-8 scans ride under the matmul, and the top-2 local candidates are
    re-scored with an exact fp32 dot product so fp32r rounding (~3.5e-7
    logit noise) cannot flip the argmax (top-2 margins go down to 5e-8).
  - GRU sharded 8 ways by gate feature columns (64 h-features per core,
    permuted weight columns r_c|z_c|n_c per core), row-major exact fp32;
    h' slices exchanged with a 32KB AllGather per stream-step.  h_old is
    selected with a one-hot matrix appended to the whh upload (SPMD
    programs cannot use per-core addresses).  PSUM zero regions are 2KB:
    exactly one start=True per bank per step.
  - Batch split into 2 streams of 128 rows; emission is interleaved per
    sub-phase so engine FIFOs never head-of-line block the other stream.
  - Cross-core argmax: (exact val, global idx) pairs, AllGather + local
    combine; ties resolve to the smallest vocab index, matching jnp.argmax.
"""
import numpy as np

VOCAB = 32002
H = 512
COND = 1024
MAXLEN = 15
B = 256
NCORES = 8
NSHARD = 4096          # per-core vocab shard (8*4096 = 32768 >= 32002)
NSTEPS = MAXLEN - 1    # 14 decode steps
P = 128
F = 64                 # h-features per core (512/8)
STREAMS = (0, 1)
KT = 4                 # hidden k-tiles (512/128)
KC = 8                 # cond k-tiles (1024/128)
NT = NSHARD // 512     # 8 pred n-tiles


def _build(bcond_nz=False, bgate_nz=False, bpred_nz=False):
    import concourse.bacc as bacc
    import concourse.mybir as mybir
    from concourse.tile import TileContext
    from concourse.bass import IndirectOffsetOnAxis

    f32 = mybir.dt.float32
    f32r = mybir.dt.float32r
    i32 = mybir.dt.int32
    u32 = mybir.dt.uint32
    AF = mybir.ActivationFunctionType
    OP = mybir.AluOpType
    AxisX = mybir.AxisListType.X

    nc = bacc.Bacc("TRN2", target_bir_lowering=False, debug=True, num_devices=NCORES)

    # ---------------- I/O ----------------
    emb = nc.declare_dram_parameter("emb", [VOCAB, H], f32, isOutput=False)
    wpt = nc.declare_dram_parameter("wpt", [H, NSHARD], f32, isOutput=False)
    wrows = nc.declare_dram_parameter("wrows", [NSHARD, H + 1], f32, isOutput=False)
    wih_r = nc.declare_dram_parameter("wih_r", [H, 3 * F], f32, isOutput=False)
    whh_r = nc.declare_dram_parameter("whh_r", [H, 3 * F + F], f32, isOutput=False)
    wct = nc.declare_dram_parameter("wct", [COND, H], f32, isOutput=False)
    imgT_d = nc.declare_dram_parameter("imgT", [COND, B], f32, isOutput=False)
    tok0 = nc.declare_dram_parameter("tok0", [B], i32, isOutput=False)
    base_t = nc.declare_dram_parameter("base_t", [P, 1], i32, isOutput=False)
    ident_in = nc.declare_dram_parameter("ident_in", [P, P], f32, isOutput=False)
    if bcond_nz:
        bcond_row = nc.declare_dram_parameter("bcond_row", [1, H], f32, isOutput=False)
    if bgate_nz:
        bgate_row = nc.declare_dram_parameter("bgate_row", [1, 4 * F], f32, isOutput=False)
    if bpred_nz:
        bpred_row = nc.declare_dram_parameter("bpred_row", [1, NSHARD], f32, isOutput=False)
    preds = nc.declare_dram_parameter("preds", [B, MAXLEN], i32, isOutput=True)

    # internal DRAM for collectives (one pair per stream-step, static)
    k_in = [[nc.dram_tensor(f"k_in_{t}_{s}", [P * 2], f32) for s in STREAMS]
            for t in range(NSTEPS)]
    k_out = [[nc.dram_tensor(f"k_out_{t}_{s}", [NCORES * P * 2], f32,
                             addr_space="Shared") for s in STREAMS]
             for t in range(NSTEPS)]
    h_in = [[nc.dram_tensor(f"h_in_{t}_{s}", [2 * P * F], f32) for s in STREAMS]
            for t in range(NSTEPS)]
    h_out = [[nc.dram_tensor(f"h_out_{t}_{s}", [NCORES * 2 * P * F], f32,
                             addr_space="Shared") for s in STREAMS]
             for t in range(NSTEPS)]

    with TileContext(nc) as tc:
        with (
            tc.tile_pool(name="wts", bufs=1) as wts,
            tc.tile_pool(name="work", bufs=1) as work,
            tc.tile_pool(name="sc", bufs=1) as sc,
            tc.tile_pool(name="psg", bufs=1, space="PSUM") as psg,
            tc.tile_pool(name="pst", bufs=1, space="PSUM") as pst,
            tc.tile_pool(name="psr", bufs=4, space="PSUM") as psr,
        ):
            # ================= resident weights =================
            base_sb = wts.tile([P, 1], i32, tag="base", name="base")
            nc.sync.dma_start(out=base_sb[:], in_=base_t[:])
            ident = wts.tile([P, P], f32, tag="ident", name="ident")
            nc.sync.dma_start(out=ident[:], in_=ident_in[:])

            wih_sb = wts.tile([P, KT * 3 * F], f32, tag="wih", name="wih")
            whh_sb = wts.tile([P, KT * 4 * F], f32, tag="whh", name="whh")
            for k in range(KT):
                nc.sync.dma_start(out=wih_sb[:, k * 3 * F:(k + 1) * 3 * F],
                                  in_=wih_r[k * P:(k + 1) * P, :])
                nc.sync.dma_start(out=whh_sb[:, k * 4 * F:(k + 1) * 4 * F],
                                  in_=whh_r[k * P:(k + 1) * P, :])

            # prediction weights: stage fp32, cast to f32r
            wpt_r = [wts.tile([P, NSHARD], f32r, tag=f"wptr{k}", name=f"wptr{k}")
                     for k in range(KT)]
            with tc.tile_pool(name="stage", bufs=2) as stage:
                for k in range(KT):
                    st = stage.tile([P, NSHARD], f32, tag="st", name="st")
                    nc.sync.dma_start(out=st[:], in_=wpt[k * P:(k + 1) * P, :])
                    nc.vector.tensor_copy(wpt_r[k][:], st[:])

            if bcond_nz or bgate_nz or bpred_nz:
                ones_col = wts.tile([1, P], f32, tag="ones", name="ones")
                nc.vector.memset(ones_col[:], 1.0)
            if bcond_nz:
                bcr_sb = wts.tile([1, H], f32, tag="bcr", name="bcr")
                nc.sync.dma_start(out=bcr_sb[:], in_=bcond_row[:])
            if bgate_nz:
                bgr_sb = wts.tile([1, 4 * F], f32, tag="bgr", name="bgr")
                nc.sync.dma_start(out=bgr_sb[:], in_=bgate_row[:])
            if bpred_nz:
                ones_col_r = wts.tile([1, P], f32r, tag="onesr", name="onesr")
                nc.vector.memset(ones_col_r[:], 1.0)
                bpr_sb = wts.tile([1, NSHARD], f32r, tag="bpr", name="bpr")
                st2 = wts.tile([1, NSHARD], f32, tag="bprs", name="bprs")
                nc.sync.dma_start(out=st2[:], in_=bpred_row[:])
                nc.vector.tensor_copy(bpr_sb[:], st2[:])

            tok_sb = [work.tile([P, 1], i32, tag=f"tok{s}", name=f"tok{s}")
                      for s in STREAMS]
            tokh = [work.tile([P, MAXLEN], i32, tag=f"tokh{s}", name=f"tokh{s}")
                    for s in STREAMS]
            for s in STREAMS:
                nc.sync.dma_start(out=tok_sb[s][:], in_=tok0[s * P:(s + 1) * P][:, None])
                nc.vector.tensor_copy(tokh[s][:, 0:1], tok_sb[s][:])

            # persistent state (h_row col H holds 1.0 for the bias-augmented
            # rescore dot)
            h_row = [work.tile([P, H + 1], f32, tag=f"hrow{s}", name=f"hrow{s}")
                     for s in STREAMS]
            hT = [work.tile([P, H], f32, tag=f"hT{s}", name=f"hT{s}")
                  for s in STREAMS]
            hTr = [work.tile([P, H], f32r, tag=f"hTr{s}", name=f"hTr{s}")
                   for s in STREAMS]
            xT = [work.tile([P, H], f32, tag=f"xT{s}", name=f"xT{s}")
                  for s in STREAMS]
            for s in STREAMS:
                nc.vector.memset(h_row[s][:, H:H + 1], 1.0)

            def transpose_to_hT(src_row, s):
                ps_tp = psr.tile([P, H], f32, tag="pred", name=f"tp{s}")
                for j in range(KT):
                    nc.tensor.transpose(ps_tp[:, j * P:(j + 1) * P],
                                        src_row[:, j * P:(j + 1) * P], ident[:])
                nc.scalar.activation(hT[s][:], ps_tp[:], AF.Copy)
                nc.vector.tensor_copy(hTr[s][:], ps_tp[:])

            # ================= h0 = img @ W_cond.T (row-major) =================
            with tc.tile_pool(name="setup", bufs=1) as setup:
                wct_sb = [setup.tile([P, H], f32, tag=f"wct{k}", name=f"wct{k}")
                          for k in range(KC)]
                imgT_sb = [setup.tile([P, B], f32, tag=f"img{k}", name=f"img{k}")
                           for k in range(KC)]
                for k in range(KC):
                    nc.sync.dma_start(out=wct_sb[k][:], in_=wct[k * P:(k + 1) * P, :])
                    nc.sync.dma_start(out=imgT_sb[k][:], in_=imgT_d[k * P:(k + 1) * P, :])
                for s in STREAMS:
                    ps_h0 = psr.tile([P, H], f32, tag="pred", name="ps_h0")
                    for k in range(KC):
                        nc.tensor.matmul(
                            ps_h0[:], lhsT=imgT_sb[k][:, s * P:(s + 1) * P],
                            rhs=wct_sb[k][:], start=(k == 0),
                            stop=(k == KC - 1 and not bcond_nz))
                    if bcond_nz:
                        nc.tensor.matmul(ps_h0[:], lhsT=ones_col[:],
                                         rhs=bcr_sb[:], start=False, stop=True)
                    nc.scalar.activation(h_row[s][:, 0:H], ps_h0[:], AF.Copy)
                    transpose_to_hT(h_row[s][:, 0:H], s)

            # ================= decode steps =================
            ps_g = [None, None]

            def emit_gh(t, s):
                ps_g[s] = psg.tile([P, 5 * F + P], f32, tag=f"g{s}", name=f"g{s}")
                pg = ps_g[s]
                for k in range(KT):
                    nc.tensor.matmul(
                        pg[:, 0:2 * F], lhsT=hT[s][:, k * P:(k + 1) * P],
                        rhs=whh_sb[:, k * 4 * F:k * 4 * F + 2 * F],
                        start=(k == 0), stop=False)
                    nc.tensor.matmul(
                        pg[:, 3 * F:5 * F], lhsT=hT[s][:, k * P:(k + 1) * P],
                        rhs=whh_sb[:, k * 4 * F + 2 * F:(k + 1) * 4 * F],
                        start=False, stop=False)

            def emit_gather_x(t, s):
                x_sb = sc.tile([P, H], f32, tag=f"x{s}", name=f"x{s}")
                nc.gpsimd.indirect_dma_start(
                    out=x_sb[:], out_offset=None, in_=emb[:],
                    in_offset=IndirectOffsetOnAxis(ap=tok_sb[s][:, :1], axis=0))
                ps_tp = pst.tile([P, H], f32, tag=f"xp{s}", name=f"tpx{s}")
                for j in range(KT):
                    nc.tensor.transpose(ps_tp[:, j * P:(j + 1) * P],
                                        x_sb[:, j * P:(j + 1) * P], ident[:])
                nc.scalar.activation(xT[s][:], ps_tp[:], AF.Copy)

            def emit_gi(t, s):
                pg = ps_g[s]
                for k in range(KT):
                    nc.tensor.matmul(
                        pg[:, 0:2 * F], lhsT=xT[s][:, k * P:(k + 1) * P],
                        rhs=wih_sb[:, k * 3 * F:k * 3 * F + 2 * F],
                        start=False, stop=False)
                    nc.tensor.matmul(
                        pg[:, 2 * F:3 * F], lhsT=xT[s][:, k * P:(k + 1) * P],
                        rhs=wih_sb[:, k * 3 * F + 2 * F:(k + 1) * 3 * F],
                        start=False, stop=(k == KT - 1 and not bgate_nz))
                if bgate_nz:
                    nc.tensor.matmul(pg[:, 0:2 * F], lhsT=ones_col[:],
                                     rhs=bgr_sb[:, 0:2 * F], start=False, stop=False)
                    nc.tensor.matmul(pg[:, 2 * F:3 * F], lhsT=ones_col[:],
                                     rhs=bgr_sb[:, 2 * F:3 * F], start=False, stop=False)
                    nc.tensor.matmul(pg[:, 3 * F:4 * F], lhsT=ones_col[:],
                                     rhs=bgr_sb[:, 3 * F:4 * F], start=False, stop=True)

            def emit_gates(t, s):
                # no vector-engine ops here: ph1 must stay off the vector FIFO
                pg = ps_g[s]
                rz_sb = sc.tile([P, 2 * F], f32, tag=f"rz{s}", name=f"rz{s}")
                nc.scalar.activation(rz_sb[:], pg[:, 0:2 * F], AF.Sigmoid)
                ih_sb = sc.tile([P, 3 * F], f32, tag=f"ih{s}", name=f"ih{s}")
                nc.scalar.activation(ih_sb[:], pg[:, 2 * F:5 * F], AF.Copy)
                t2_sb = sc.tile([P, F], f32, tag=f"t2{s}", name=f"t2{s}")
                nc.gpsimd.tensor_mul(t2_sb[:], rz_sb[:, 0:F], ih_sb[:, F:2 * F])
                nc.gpsimd.tensor_add(t2_sb[:], t2_sb[:], ih_sb[:, 0:F])
                n_sb = sc.tile([P, F], f32, tag=f"n{s}", name=f"n{s}")
                nc.scalar.activation(n_sb[:], t2_sb[:], AF.Tanh)
                # h' = n + z*(h_old - n)
                d_sb = sc.tile([P, F], f32, tag=f"d{s}", name=f"d{s}")
                nc.gpsimd.tensor_sub(d_sb[:], ih_sb[:, 2 * F:3 * F], n_sb[:])
                nc.gpsimd.tensor_mul(d_sb[:], d_sb[:], rz_sb[:, F:2 * F])
                nc.gpsimd.tensor_add(d_sb[:], d_sb[:], n_sb[:])
                # transposed copy of the slice so hT can be assembled by DMA
                # (parked in the spare columns of the gates psum bank)
                nc.tensor.transpose(pg[0:F, 5 * F:5 * F + P], d_sb[:], ident[:])
                dT_sb = sc.tile([F, P], f32, tag=f"dT{s}", name=f"dT{s}")
                nc.scalar.activation(dT_sb[:], pg[0:F, 5 * F:5 * F + P], AF.Copy)
                return d_sb, dT_sb

            def emit_hshare(t, s, d_pair):
                d_sb, dT_sb = d_pair
                nc.gpsimd.dma_start(
                    out=h_in[t][s][0:P * F].rearrange("(p f) -> p f", f=F),
                    in_=d_sb[:])
                nc.gpsimd.dma_start(
                    out=h_in[t][s][P * F:2 * P * F].rearrange("(f p) -> f p", p=P),
                    in_=dT_sb[:])
                nc.gpsimd.collective_compute(
                    "AllGather", OP.bypass,
                    replica_groups=[list(range(NCORES))],
                    ins=[h_in[t][s][:]], outs=[h_out[t][s][:]])

            def emit_hback(t, s):
                hv = h_out[t][s][:].rearrange("(c r) -> c r", r=2 * P * F)
                # row-major part (per-core [p, f] blocks)
                nc.sync.dma_start(
                    out=h_row[s][:, 0:H].rearrange("p (c f) -> p c f", f=F),
                    in_=hv[:, 0:P * F].rearrange("c (p f) -> p c f", f=F))
                # transposed part (per-core [f, p] blocks) -> hT directly,
                # one DMA per partition half (partition dim cannot split)
                tv = hv[:, P * F:2 * P * F].rearrange(
                    "(c2 cl) (f p) -> cl f c2 p", cl=2, p=P)
                for cl in range(2):
                    nc.sync.dma_start(
                        out=hT[s][cl * F:(cl + 1) * F, :].rearrange(
                            "f (c2 p) -> f c2 p", c2=4),
                        in_=tv[cl])
                nc.scalar.activation(hTr[s][:], hT[s][:], AF.Copy)

            def emit_pred(t, s):
                """fp32r pred with per-block top-8 scans riding under it."""
                bt2 = sc.tile([P, 2 * NT], f32, tag=f"bt2{s}", name=f"bt2{s}")
                bi2 = sc.tile([P, 2 * NT], i32, tag=f"bi2{s}", name=f"bi2{s}")
                for n in range(NT):
                    ps_pred = psr.tile([P, 512], f32, tag="pred", name="pred")
                    for k in range(KT):
                        nc.tensor.matmul(
                            ps_pred[:], lhsT=hTr[s][:, k * P:(k + 1) * P],
                            rhs=wpt_r[k][:, n * 512:(n + 1) * 512],
                            start=(k == 0),
                            stop=(k == KT - 1 and not bpred_nz))
                    if bpred_nz:
                        nc.tensor.matmul(
                            ps_pred[:], lhsT=ones_col_r[:],
                            rhs=bpr_sb[:, n * 512:(n + 1) * 512],
                            start=False, stop=True)
                    bm = sc.tile([P, 8], f32, tag=f"bm{s}_{n % 2}", name=f"bm{s}_{n}")
                    bi = sc.tile([P, 8], u32, tag=f"bi{s}_{n % 2}", name=f"bi{s}_{n}")
                    nc.vector.max(out=bm[:], in_=ps_pred[:])
                    nc.vector.max_index(out=bi[:], in_max=bm[:], in_values=ps_pred[:])
                    nc.vector.tensor_copy(bt2[:, 2 * n:2 * n + 2], bm[:, 0:2])
                    nc.vector.tensor_scalar_add(bi2[:, 2 * n:2 * n + 2],
                                                bi[:, 0:2].bitcast(i32), n * 512)
                return bt2, bi2

            def emit_argmax_rescore(t, s, bt2, bi2):
                # local top-2 across the 8 blocks; launch gathers eagerly
                g8 = sc.tile([P, 8], f32, tag=f"g8{s}", name=f"g8{s}")
                nc.vector.max(out=g8[:], in_=bt2[:])
                li = sc.tile([P, 2], i32, tag=f"li{s}", name=f"li{s}")
                w1 = sc.tile([P, H + 1], f32, tag=f"w1{s}", name=f"w1{s}")
                w2 = sc.tile([P, H + 1], f32, tag=f"w2{s}", name=f"w2{s}")
                for j, wj in ((0, w1), (1, w2)):
                    msk = sc.tile([P, 2 * NT], u32, tag=f"msk{s}", name=f"msk{s}")
                    nc.vector.tensor_tensor(msk[:], bt2[:],
                                            g8[:, j:j + 1].to_broadcast([P, 2 * NT]),
                                            OP.is_ge)
                    cnd = sc.tile([P, 2 * NT], i32, tag=f"cnd{s}", name=f"cnd{s}")
                    nc.vector.memset(cnd[:], 0x7FFFFFFF)
                    nc.vector.copy_predicated(cnd[:], msk[:], bi2[:])
                    nc.vector.tensor_reduce(li[:, j:j + 1], cnd[:], AxisX, OP.min)
                    nc.gpsimd.indirect_dma_start(
                        out=wj[:], out_offset=None, in_=wrows[:],
                        in_offset=IndirectOffsetOnAxis(ap=li[:, j:j + 1], axis=0))
                nc.vector.tensor_mul(w1[:], w1[:], h_row[s][:])
                nc.vector.tensor_mul(w2[:], w2[:], h_row[s][:])
                ev = sc.tile([P, 2], f32, tag=f"ev{s}", name=f"ev{s}")
                nc.vector.tensor_reduce(ev[:, 0:1], w1[:], AxisX, OP.add)
                nc.vector.tensor_reduce(ev[:, 1:2], w2[:], AxisX, OP.add)
                # key = (max exact val, its global idx; tie -> min idx)
                gi2 = sc.tile([P, 2], i32, tag=f"gi2{s}", name=f"gi2{s}")
                nc.vector.tensor_add(gi2[:, 0:1], li[:, 0:1], base_sb[:])
                nc.vector.tensor_add(gi2[:, 1:2], li[:, 1:2], base_sb[:])
                key = sc.tile([P, 2], f32, tag=f"key{s}", name=f"key{s}")
                nc.vector.tensor_reduce(key[:, 0:1], ev[:], AxisX, OP.max)
                mk2 = sc.tile([P, 2], u32, tag=f"mk2{s}", name=f"mk2{s}")
                nc.vector.tensor_tensor(mk2[:], ev[:],
                                        key[:, 0:1].to_broadcast([P, 2]), OP.is_ge)
                cn2 = sc.tile([P, 2], i32, tag=f"cn2{s}", name=f"cn2{s}")
                nc.vector.memset(cn2[:], 0x7FFFFFFF)
                nc.vector.copy_predicated(cn2[:], mk2[:], gi2[:])
                nc.vector.tensor_reduce(key[:, 1:2].bitcast(i32), cn2[:], AxisX, OP.min)
                nc.gpsimd.dma_start(
                    out=k_in[t][s][:].rearrange("(p w) -> p w", w=2),
                    in_=key[:])
                nc.gpsimd.collective_compute(
                    "AllGather", OP.bypass,
                    replica_groups=[list(range(NCORES))],
                    ins=[k_in[t][s][:]], outs=[k_out[t][s][:]])

            def emit_combine(t, s):
                # entirely on gpsimd: chains into the gpsimd x-gather with no
                # cross-engine hop, and stays off the congested vector FIFO
                kv = sc.tile([P, 2 * NCORES], f32, tag=f"kv{s}", name=f"kv{s}")
                nc.sync.dma_start(
                    out=kv[:].rearrange("p (c w) -> p c w", w=2),
                    in_=k_out[t][s][:].rearrange("(c p w) -> p c w", c=NCORES, w=2))
                vals8 = kv[:].rearrange("p (c w) -> p c w", w=2)[:, :, 0]
                idx8 = kv[:].rearrange("p (c w) -> p c w", w=2)[:, :, 1].bitcast(i32)
                gmax = sc.tile([P, 1], f32, tag=f"gm{s}", name=f"gm{s}")
                nc.vector.tensor_reduce(gmax[:], vals8, AxisX, OP.max)
                mask = sc.tile([P, NCORES], u32, tag=f"mk{s}", name=f"mk{s}")
                nc.vector.tensor_tensor(mask[:], vals8,
                                        gmax[:].to_broadcast([P, NCORES]), OP.is_ge)
                cand = sc.tile([P, NCORES], i32, tag=f"cd{s}", name=f"cd{s}")
                nc.vector.memset(cand[:], 0x7FFFFFFF)
                nc.vector.copy_predicated(cand[:], mask[:], idx8)
                tok_new = work.tile([P, 1], i32, tag=f"tok{s}", name=f"tok{s}")
                nc.vector.tensor_reduce(tok_new[:], cand[:], AxisX, OP.min)
                tok_sb[s] = tok_new
                nc.vector.tensor_copy(tokh[s][:, t + 1:t + 2], tok_new[:])

            def ph1(t, s):
                """GRU half: tok(t) -> h'(t) -> coll2 launch.  No vector ops."""
                emit_gh(t, s)
                emit_gather_x(t, s)
                emit_gi(t, s)
                d_pair = emit_gates(t, s)
                emit_hshare(t, s, d_pair)

            bt_c = [None, None]

            def ph2a(t, s):
                """h(t) back -> logits + riding block scans."""
                emit_hback(t, s)
                bt_c[s] = emit_pred(t, s)

            def ph2b(t, s):
                """top-2 select -> exact rescore -> key -> coll1 launch."""
                emit_argmax_rescore(t, s, *bt_c[s])

            # software-pipelined half-phase schedule, emitted in expected
            # execution order so no engine FIFO head-of-line blocks the
            # other stream
            ph1(0, 0)
            for t in range(NSTEPS):
                ph2a(t, 0)
                ph1(t, 1)
                ph2b(t, 0)
                emit_combine(t, 0)
                ph2a(t, 1)
                if t + 1 < NSTEPS:
                    ph1(t + 1, 0)
                ph2b(t, 1)
                emit_combine(t, 1)

            # final: write predictions once per stream
            for s in STREAMS:
                nc.sync.dma_start(out=preds[s * P:(s + 1) * P, :], in_=tokh[s][:])

    return nc


def _prep_inputs(caption, img, embedding, W_cond, b_cond, w_ih, w_hh, b_ih,
                 b_hh, W_pred, b_pred):
    caption = np.asarray(caption).astype(np.int32)
    img = np.ascontiguousarray(np.asarray(img, dtype=np.float32))
    embedding = np.ascontiguousarray(np.asarray(embedding, dtype=np.float32))
    W_pred = np.asarray(W_pred, dtype=np.float32)
    b_pred = np.asarray(b_pred, np.float32)
    b_ih = np.asarray(b_ih, np.float32)
    b_hh = np.asarray(b_hh, np.float32)
    wihT = np.ascontiguousarray(np.asarray(w_ih, np.float32).T)   # [H, 3H]
    whhT = np.ascontiguousarray(np.asarray(w_hh, np.float32).T)
    common = dict(
        emb=embedding,
        wct=np.ascontiguousarray(np.asarray(W_cond, np.float32).T),
        imgT=np.ascontiguousarray(img.T),
        tok0=np.ascontiguousarray(caption[:, 0]),
        ident_in=np.eye(P, dtype=np.float32),
        bcond_row=np.asarray(b_cond, np.float32).reshape(1, H),
    )
    in_maps = []
    for c in range(NCORES):
        base = c * NSHARD
        hi = min(base + NSHARD, VOCAB)
        n_real = max(0, hi - base)
        wpt_c = np.zeros((H, NSHARD), np.float32)
        wpt_c[:, :n_real] = W_pred[base:hi].T
        wrows_c = np.zeros((NSHARD, H + 1), np.float32)
        wrows_c[:n_real, :H] = W_pred[base:hi]
        wrows_c[:n_real, H] = b_pred[base:hi]
        lo, hi_f = c * F, (c + 1) * F
        wih_rc = np.concatenate(
            [wihT[:, lo:hi_f], wihT[:, H + lo:H + hi_f],
             wihT[:, 2 * H + lo:2 * H + hi_f]], axis=1)
        sel = np.zeros((H, F), np.float32)
        sel[lo:hi_f, :] = np.eye(F, dtype=np.float32)
        whh_rc = np.concatenate(
            [whhT[:, lo:hi_f], whhT[:, H + lo:H + hi_f],
             whhT[:, 2 * H + lo:2 * H + hi_f], sel], axis=1)
        bg = b_ih + b_hh
        bgate_row = np.concatenate(
            [bg[lo:hi_f], bg[H + lo:H + hi_f],
             b_ih[2 * H + lo:2 * H + hi_f], b_hh[2 * H + lo:2 * H + hi_f]])
        m = dict(common)
        m["wpt"] = np.ascontiguousarray(wpt_c)
        m["wrows"] = np.ascontiguousarray(wrows_c)
        m["wih_r"] = np.ascontiguousarray(wih_rc)
        m["whh_r"] = np.ascontiguousarray(whh_rc)
        m["base_t"] = np.full((P, 1), base, np.int32)
        m["bgate_row"] = np.ascontiguousarray(bgate_row.reshape(1, 4 * F))
        bp = np.zeros((1, NSHARD), np.float32)
        bp[0, :n_real] = b_pred[base:hi]
        m["bpred_row"] = bp
        in_maps.append(m)
    return in_maps


_CACHED = {}


def kernel(**inputs) -> np.ndarray:
    from concourse.bass_utils import run_bass_kernel_spmd

    in_maps = _prep_inputs(**inputs)
    bcond_nz = bool(np.any(np.asarray(inputs["b_cond"])))
    bgate_nz = bool(np.any(np.asarray(inputs["b_ih"]))
                    or np.any(np.asarray(inputs["b_hh"])))
    bpred_nz = bool(np.any(np.asarray(inputs["b_pred"])))
    key = (bcond_nz, bgate_nz, bpred_nz)
    if key not in _CACHED:
        nc = _build(*key)
        nc.finalize()
        _CACHED[key] = nc
    flags = ("bcond_row", "bgate_row", "bpred_row")
    drop = [f for f, nz in zip(flags, key) if not nz]
    for m in in_maps:
        for f in drop:
            m.pop(f, None)
    res = run_bass_kernel_spmd(_CACHED[key], in_maps, list(range(NCORES)))
    return np.ascontiguousarray(res.results[0]["preds"].astype(np.int32))


if __name__ == "__main__":
    d = np.load("inputs.npz")
    inputs = {k: d[k] for k in d.files}
    out = kernel(**inputs)
    exp = np.load("expected.npy")
    print("match:", np.array_equal(out, exp),
          " mismatches:", int((out != exp).sum()), "/", out.size)


# revision 25
# speedup vs baseline: 1.1196x; 1.0990x over previous
"""ConditionalLM decode kernel for 8 Trainium2 NeuronCores.

Strategy (v4):
  - Vocab-shard W_pred across 8 cores (4096 cols each, zero-padded); the
    prediction matmul runs in float32r (1 cyc/row vs 4 for fp32); per-block
    top-8 scans ride under the matmul, and the top-2 local candidates are
    re-scored with an exact fp32 dot product so fp32r rounding (~3.5e-7
    logit noise) cannot flip the argmax (top-2 margins go down to 5e-8).
  - GRU sharded 8 ways by gate feature columns (64 h-features per core,
    permuted weight columns r_c|z_c|n_c per core), row-major exact fp32;
    h' slices exchanged with a 32KB AllGather per stream-step.  h_old is
    selected with a one-hot matrix appended to the whh upload (SPMD
    programs cannot use per-core addresses).  PSUM zero regions are 2KB:
    exactly one start=True per bank per step.
  - Batch split into 2 streams of 128 rows; emission is interleaved per
    sub-phase so engine FIFOs never head-of-line block the other stream.
  - Cross-core argmax: (exact val, global idx) pairs, AllGather + local
    combine; ties resolve to the smallest vocab index, matching jnp.argmax.
"""
import numpy as np

VOCAB = 32002
H = 512
COND = 1024
MAXLEN = 15
B = 256
NCORES = 8
NSHARD = 4096          # per-core vocab shard (8*4096 = 32768 >= 32002)
NSTEPS = MAXLEN - 1    # 14 decode steps
P = 128
F = 64                 # h-features per core (512/8)
STREAMS = (0, 1)
KT = 4                 # hidden k-tiles (512/128)
KC = 8                 # cond k-tiles (1024/128)
NT = NSHARD // 512     # 8 pred n-tiles


def _build(bcond_nz=False, bgate_nz=False, bpred_nz=False):
    import concourse.bacc as bacc
    import concourse.mybir as mybir
    from concourse.tile import TileContext
    from concourse.bass import IndirectOffsetOnAxis

    f32 = mybir.dt.float32
    f32r = mybir.dt.float32r
    i32 = mybir.dt.int32
    u32 = mybir.dt.uint32
    AF = mybir.ActivationFunctionType
    OP = mybir.AluOpType
    AxisX = mybir.AxisListType.X

    nc = bacc.Bacc("TRN2", target_bir_lowering=False, debug=True, num_devices=NCORES)

    # ---------------- I/O ----------------
    emb = nc.declare_dram_parameter("emb", [VOCAB, H], f32, isOutput=False)
    wpt = nc.declare_dram_parameter("wpt", [H, NSHARD], f32, isOutput=False)
    wrows = nc.declare_dram_parameter("wrows", [NSHARD, H + 1], f32, isOutput=False)
    wih_r = nc.declare_dram_parameter("wih_r", [H, 3 * F], f32, isOutput=False)
    whh_r = nc.declare_dram_parameter("whh_r", [H, 3 * F + F], f32, isOutput=False)
    wct = nc.declare_dram_parameter("wct", [COND, H], f32, isOutput=False)
    imgT_d = nc.declare_dram_parameter("imgT", [COND, B], f32, isOutput=False)
    tok0 = nc.declare_dram_parameter("tok0", [B], i32, isOutput=False)
    base_t = nc.declare_dram_parameter("base_t", [P, 1], i32, isOutput=False)
    ident_in = nc.declare_dram_parameter("ident_in", [P, P], f32, isOutput=False)
    if bcond_nz:
        bcond_row = nc.declare_dram_parameter("bcond_row", [1, H], f32, isOutput=False)
    if bgate_nz:
        bgate_row = nc.declare_dram_parameter("bgate_row", [1, 4 * F], f32, isOutput=False)
    if bpred_nz:
        bpred_row = nc.declare_dram_parameter("bpred_row", [1, NSHARD], f32, isOutput=False)
    preds = nc.declare_dram_parameter("preds", [B, MAXLEN], i32, isOutput=True)

    # internal DRAM for collectives (one pair per stream-step, static)
    k_in = [[nc.dram_tensor(f"k_in_{t}_{s}", [P * 2], f32) for s in STREAMS]
            for t in range(NSTEPS)]
    k_out = [[nc.dram_tensor(f"k_out_{t}_{s}", [NCORES * P * 2], f32,
                             addr_space="Shared") for s in STREAMS]
             for t in range(NSTEPS)]
    h_in = [[nc.dram_tensor(f"h_in_{t}_{s}", [2 * P * F], f32) for s in STREAMS]
            for t in range(NSTEPS)]
    h_out = [[nc.dram_tensor(f"h_out_{t}_{s}", [NCORES * 2 * P * F], f32,
                             addr_space="Shared") for s in STREAMS]
             for t in range(NSTEPS)]

    with TileContext(nc) as tc:
        with (
            tc.tile_pool(name="wts", bufs=1) as wts,
            tc.tile_pool(name="work", bufs=1) as work,
            tc.tile_pool(name="sc", bufs=1) as sc,
            tc.tile_pool(name="psg", bufs=1, space="PSUM") as psg,
            tc.tile_pool(name="pst", bufs=1, space="PSUM") as pst,
            tc.tile_pool(name="psr", bufs=4, space="PSUM") as psr,
        ):
            # ================= resident weights =================
            base_sb = wts.tile([P, 1], i32, tag="base", name="base")
            nc.sync.dma_start(out=base_sb[:], in_=base_t[:])
            ident = wts.tile([P, P], f32, tag="ident", name="ident")
            nc.sync.dma_start(out=ident[:], in_=ident_in[:])

            wih_sb = wts.tile([P, KT * 3 * F], f32, tag="wih", name="wih")
            whh_sb = wts.tile([P, KT * 4 * F], f32, tag="whh", name="whh")
            for k in range(KT):
                nc.sync.dma_start(out=wih_sb[:, k * 3 * F:(k + 1) * 3 * F],
                                  in_=wih_r[k * P:(k + 1) * P, :])
                nc.sync.dma_start(out=whh_sb[:, k * 4 * F:(k + 1) * 4 * F],
                                  in_=whh_r[k * P:(k + 1) * P, :])

            # prediction weights: stage fp32, cast to f32r
            wpt_r = [wts.tile([P, NSHARD], f32r, tag=f"wptr{k}", name=f"wptr{k}")
                     for k in range(KT)]
            with tc.tile_pool(name="stage", bufs=2) as stage:
                for k in range(KT):
                    st = stage.tile([P, NSHARD], f32, tag="st", name="st")
                    nc.sync.dma_start(out=st[:], in_=wpt[k * P:(k + 1) * P, :])
                    nc.vector.tensor_copy(wpt_r[k][:], st[:])

            if bcond_nz or bgate_nz or bpred_nz:
                ones_col = wts.tile([1, P], f32, tag="ones", name="ones")
                nc.vector.memset(ones_col[:], 1.0)
            if bcond_nz:
                bcr_sb = wts.tile([1, H], f32, tag="bcr", name="bcr")
                nc.sync.dma_start(out=bcr_sb[:], in_=bcond_row[:])
            if bgate_nz:
                bgr_sb = wts.tile([1, 4 * F], f32, tag="bgr", name="bgr")
                nc.sync.dma_start(out=bgr_sb[:], in_=bgate_row[:])
            if bpred_nz:
                ones_col_r = wts.tile([1, P], f32r, tag="onesr", name="onesr")
                nc.vector.memset(ones_col_r[:], 1.0)
                bpr_sb = wts.tile([1, NSHARD], f32r, tag="bpr", name="bpr")
                st2 = wts.tile([1, NSHARD], f32, tag="bprs", name="bprs")
                nc.sync.dma_start(out=st2[:], in_=bpred_row[:])
                nc.vector.tensor_copy(bpr_sb[:], st2[:])

            tok_sb = [work.tile([P, 1], i32, tag=f"tok{s}", name=f"tok{s}")
                      for s in STREAMS]
            tokh = [work.tile([P, MAXLEN], i32, tag=f"tokh{s}", name=f"tokh{s}")
                    for s in STREAMS]
            for s in STREAMS:
                nc.sync.dma_start(out=tok_sb[s][:], in_=tok0[s * P:(s + 1) * P][:, None])
                nc.vector.tensor_copy(tokh[s][:, 0:1], tok_sb[s][:])

            # persistent state (h_row col H holds 1.0 for the bias-augmented
            # rescore dot)
            h_row = [work.tile([P, H + 1], f32, tag=f"hrow{s}", name=f"hrow{s}")
                     for s in STREAMS]
            hT = [work.tile([P, H], f32, tag=f"hT{s}", name=f"hT{s}")
                  for s in STREAMS]
            hTr = [work.tile([P, H], f32r, tag=f"hTr{s}", name=f"hTr{s}")
                   for s in STREAMS]
            xT = [work.tile([P, H], f32, tag=f"xT{s}", name=f"xT{s}")
                  for s in STREAMS]
            for s in STREAMS:
                nc.vector.memset(h_row[s][:, H:H + 1], 1.0)

            def transpose_to_hT(src_row, s):
                ps_tp = psr.tile([P, H], f32, tag="pred", name=f"tp{s}")
                for j in range(KT):
                    nc.tensor.transpose(ps_tp[:, j * P:(j + 1) * P],
                                        src_row[:, j * P:(j + 1) * P], ident[:])
                nc.scalar.activation(hT[s][:], ps_tp[:], AF.Copy)
                nc.vector.tensor_copy(hTr[s][:], ps_tp[:])

            # ================= h0 = img @ W_cond.T (row-major) =================
            with tc.tile_pool(name="setup", bufs=1) as setup:
                wct_sb = [setup.tile([P, H], f32, tag=f"wct{k}", name=f"wct{k}")
                          for k in range(KC)]
                imgT_sb = [setup.tile([P, B], f32, tag=f"img{k}", name=f"img{k}")
                           for k in range(KC)]
                for k in range(KC):
                    nc.sync.dma_start(out=wct_sb[k][:], in_=wct[k * P:(k + 1) * P, :])
                    nc.sync.dma_start(out=imgT_sb[k][:], in_=imgT_d[k * P:(k + 1) * P, :])
                for s in STREAMS:
                    ps_h0 = psr.tile([P, H], f32, tag="pred", name="ps_h0")
                    for k in range(KC):
                        nc.tensor.matmul(
                            ps_h0[:], lhsT=imgT_sb[k][:, s * P:(s + 1) * P],
                            rhs=wct_sb[k][:], start=(k == 0),
                            stop=(k == KC - 1 and not bcond_nz))
                    if bcond_nz:
                        nc.tensor.matmul(ps_h0[:], lhsT=ones_col[:],
                                         rhs=bcr_sb[:], start=False, stop=True)
                    nc.scalar.activation(h_row[s][:, 0:H], ps_h0[:], AF.Copy)
                    transpose_to_hT(h_row[s][:, 0:H], s)

            # ================= decode steps =================
            ps_g = [None, None]

            def emit_gh(t, s):
                ps_g[s] = psg.tile([P, 5 * F + P], f32, tag=f"g{s}", name=f"g{s}")
                pg = ps_g[s]
                for k in range(KT):
                    nc.tensor.matmul(
                        pg[:, 0:2 * F], lhsT=hT[s][:, k * P:(k + 1) * P],
                        rhs=whh_sb[:, k * 4 * F:k * 4 * F + 2 * F],
                        start=(k == 0), stop=False)
                    nc.tensor.matmul(
                        pg[:, 3 * F:5 * F], lhsT=hT[s][:, k * P:(k + 1) * P],
                        rhs=whh_sb[:, k * 4 * F + 2 * F:(k + 1) * 4 * F],
                        start=False, stop=False)

            def emit_gather_x(t, s):
                x_sb = sc.tile([P, H], f32, tag=f"x{s}", name=f"x{s}")
                nc.gpsimd.indirect_dma_start(
                    out=x_sb[:], out_offset=None, in_=emb[:],
                    in_offset=IndirectOffsetOnAxis(ap=tok_sb[s][:, :1], axis=0))
                ps_tp = pst.tile([P, H], f32, tag=f"xp{s}", name=f"tpx{s}")
                for j in range(KT):
                    nc.tensor.transpose(ps_tp[:, j * P:(j + 1) * P],
                                        x_sb[:, j * P:(j + 1) * P], ident[:])
                nc.scalar.activation(xT[s][:], ps_tp[:], AF.Copy)

            def emit_gi(t, s):
                pg = ps_g[s]
                for k in range(KT):
                    nc.tensor.matmul(
                        pg[:, 0:2 * F], lhsT=xT[s][:, k * P:(k + 1) * P],
                        rhs=wih_sb[:, k * 3 * F:k * 3 * F + 2 * F],
                        start=False, stop=False)
                    nc.tensor.matmul(
                        pg[:, 2 * F:3 * F], lhsT=xT[s][:, k * P:(k + 1) * P],
                        rhs=wih_sb[:, k * 3 * F + 2 * F:(k + 1) * 3 * F],
                        start=False, stop=(k == KT - 1 and not bgate_nz))
                if bgate_nz:
                    nc.tensor.matmul(pg[:, 0:2 * F], lhsT=ones_col[:],
                                     rhs=bgr_sb[:, 0:2 * F], start=False, stop=False)
                    nc.tensor.matmul(pg[:, 2 * F:3 * F], lhsT=ones_col[:],
                                     rhs=bgr_sb[:, 2 * F:3 * F], start=False, stop=False)
                    nc.tensor.matmul(pg[:, 3 * F:4 * F], lhsT=ones_col[:],
                                     rhs=bgr_sb[:, 3 * F:4 * F], start=False, stop=True)

            def emit_gates(t, s):
                # no vector-engine ops here: ph1 must stay off the vector FIFO
                pg = ps_g[s]
                rz_sb = sc.tile([P, 2 * F], f32, tag=f"rz{s}", name=f"rz{s}")
                nc.scalar.activation(rz_sb[:], pg[:, 0:2 * F], AF.Sigmoid)
                ih_sb = sc.tile([P, 3 * F], f32, tag=f"ih{s}", name=f"ih{s}")
                nc.scalar.activation(ih_sb[:], pg[:, 2 * F:5 * F], AF.Copy)
                t2_sb = sc.tile([P, F], f32, tag=f"t2{s}", name=f"t2{s}")
                nc.gpsimd.tensor_mul(t2_sb[:], rz_sb[:, 0:F], ih_sb[:, F:2 * F])
                nc.gpsimd.tensor_add(t2_sb[:], t2_sb[:], ih_sb[:, 0:F])
                n_sb = sc.tile([P, F], f32, tag=f"n{s}", name=f"n{s}")
                nc.scalar.activation(n_sb[:], t2_sb[:], AF.Tanh)
                # h' = n + z*(h_old - n)
                d_sb = sc.tile([P, F], f32, tag=f"d{s}", name=f"d{s}")
                nc.gpsimd.tensor_sub(d_sb[:], ih_sb[:, 2 * F:3 * F], n_sb[:])
                nc.gpsimd.tensor_mul(d_sb[:], d_sb[:], rz_sb[:, F:2 * F])
                nc.gpsimd.tensor_add(d_sb[:], d_sb[:], n_sb[:])
                # transposed copy of the slice so hT can be assembled by DMA
                # (parked in the spare columns of the gates psum bank)
                nc.tensor.transpose(pg[0:F, 5 * F:5 * F + P], d_sb[:], ident[:])
                dT_sb = sc.tile([F, P], f32, tag=f"dT{s}", name=f"dT{s}")
                nc.scalar.activation(dT_sb[:], pg[0:F, 5 * F:5 * F + P], AF.Copy)
                return d_sb, dT_sb

            def emit_hshare(t, s, d_pair):
                d_sb, dT_sb = d_pair
                nc.sync.dma_start(
                    out=h_in[t][s][0:P * F].rearrange("(p f) -> p f", f=F),
                    in_=d_sb[:])
                nc.sync.dma_start(
                    out=h_in[t][s][P * F:2 * P * F].rearrange("(f p) -> f p", p=P),
                    in_=dT_sb[:])
                nc.gpsimd.collective_compute(
                    "AllGather", OP.bypass,
                    replica_groups=[list(range(NCORES))],
                    ins=[h_in[t][s][:]], outs=[h_out[t][s][:]])

            def emit_hback(t, s):
                hv = h_out[t][s][:].rearrange("(c r) -> c r", r=2 * P * F)
                # row-major part (per-core [p, f] blocks)
                nc.sync.dma_start(
                    out=h_row[s][:, 0:H].rearrange("p (c f) -> p c f", f=F),
                    in_=hv[:, 0:P * F].rearrange("c (p f) -> p c f", f=F))
                # transposed part (per-core [f, p] blocks) -> hT directly,
                # one DMA per partition half (partition dim cannot split)
                tv = hv[:, P * F:2 * P * F].rearrange(
                    "(c2 cl) (f p) -> cl f c2 p", cl=2, p=P)
                for cl in range(2):
                    nc.sync.dma_start(
                        out=hT[s][cl * F:(cl + 1) * F, :].rearrange(
                            "f (c2 p) -> f c2 p", c2=4),
                        in_=tv[cl])
                nc.scalar.activation(hTr[s][:], hT[s][:], AF.Copy)

            def emit_pred(t, s):
                """fp32r pred with per-block top-8 scans riding under it."""
                bt2 = sc.tile([P, 2 * NT], f32, tag=f"bt2{s}", name=f"bt2{s}")
                bi2 = sc.tile([P, 2 * NT], i32, tag=f"bi2{s}", name=f"bi2{s}")
                for n in range(NT):
                    ps_pred = psr.tile([P, 512], f32, tag="pred", name="pred")
                    for k in range(KT):
                        nc.tensor.matmul(
                            ps_pred[:], lhsT=hTr[s][:, k * P:(k + 1) * P],
                            rhs=wpt_r[k][:, n * 512:(n + 1) * 512],
                            start=(k == 0),
                            stop=(k == KT - 1 and not bpred_nz))
                    if bpred_nz:
                        nc.tensor.matmul(
                            ps_pred[:], lhsT=ones_col_r[:],
                            rhs=bpr_sb[:, n * 512:(n + 1) * 512],
                            start=False, stop=True)
                    bm = sc.tile([P, 8], f32, tag=f"bm{s}_{n % 2}", name=f"bm{s}_{n}")
                    bi = sc.tile([P, 8], u32, tag=f"bi{s}_{n % 2}", name=f"bi{s}_{n}")
                    nc.vector.max(out=bm[:], in_=ps_pred[:])
                    nc.vector.max_index(out=bi[:], in_max=bm[:], in_values=ps_pred[:])
                    nc.vector.tensor_copy(bt2[:, 2 * n:2 * n + 2], bm[:, 0:2])
                    nc.vector.tensor_scalar_add(bi2[:, 2 * n:2 * n + 2],
                                                bi[:, 0:2].bitcast(i32), n * 512)
                return bt2, bi2

            def emit_argmax_rescore(t, s, bt2, bi2):
                # local top-2 across the 8 blocks; launch gathers eagerly
                g8 = sc.tile([P, 8], f32, tag=f"g8{s}", name=f"g8{s}")
                nc.vector.max(out=g8[:], in_=bt2[:])
                li = sc.tile([P, 2], i32, tag=f"li{s}", name=f"li{s}")
                w1 = sc.tile([P, H + 1], f32, tag=f"w1{s}", name=f"w1{s}")
                w2 = sc.tile([P, H + 1], f32, tag=f"w2{s}", name=f"w2{s}")
                for j, wj in ((0, w1), (1, w2)):
                    msk = sc.tile([P, 2 * NT], u32, tag=f"msk{s}", name=f"msk{s}")
                    nc.vector.tensor_tensor(msk[:], bt2[:],
                                            g8[:, j:j + 1].to_broadcast([P, 2 * NT]),
                                            OP.is_ge)
                    cnd = sc.tile([P, 2 * NT], i32, tag=f"cnd{s}", name=f"cnd{s}")
                    nc.vector.memset(cnd[:], 0x7FFFFFFF)
                    nc.vector.copy_predicated(cnd[:], msk[:], bi2[:])
                    nc.vector.tensor_reduce(li[:, j:j + 1], cnd[:], AxisX, OP.min)
                    nc.gpsimd.indirect_dma_start(
                        out=wj[:], out_offset=None, in_=wrows[:],
                        in_offset=IndirectOffsetOnAxis(ap=li[:, j:j + 1], axis=0))
                nc.vector.tensor_mul(w1[:], w1[:], h_row[s][:])
                nc.vector.tensor_mul(w2[:], w2[:], h_row[s][:])
                ev = sc.tile([P, 2], f32, tag=f"ev{s}", name=f"ev{s}")
                nc.vector.tensor_reduce(ev[:, 0:1], w1[:], AxisX, OP.add)
                nc.vector.tensor_reduce(ev[:, 1:2], w2[:], AxisX, OP.add)
                # key = (max exact val, its global idx; tie -> min idx)
                gi2 = sc.tile([P, 2], i32, tag=f"gi2{s}", name=f"gi2{s}")
                nc.vector.tensor_add(gi2[:, 0:1], li[:, 0:1], base_sb[:])
                nc.vector.tensor_add(gi2[:, 1:2], li[:, 1:2], base_sb[:])
                key = sc.tile([P, 2], f32, tag=f"key{s}", name=f"key{s}")
                nc.vector.tensor_reduce(key[:, 0:1], ev[:], AxisX, OP.max)
                mk2 = sc.tile([P, 2], u32, tag=f"mk2{s}", name=f"mk2{s}")
                nc.vector.tensor_tensor(mk2[:], ev[:],
                                        key[:, 0:1].to_broadcast([P, 2]), OP.is_ge)
                cn2 = sc.tile([P, 2], i32, tag=f"cn2{s}", name=f"cn2{s}")
                nc.vector.memset(cn2[:], 0x7FFFFFFF)
                nc.vector.copy_predicated(cn2[:], mk2[:], gi2[:])
                nc.vector.tensor_reduce(key[:, 1:2].bitcast(i32), cn2[:], AxisX, OP.min)
                nc.sync.dma_start(
                    out=k_in[t][s][:].rearrange("(p w) -> p w", w=2),
                    in_=key[:])
                nc.gpsimd.collective_compute(
                    "AllGather", OP.bypass,
                    replica_groups=[list(range(NCORES))],
                    ins=[k_in[t][s][:]], outs=[k_out[t][s][:]])

            def emit_combine(t, s):
                # entirely on gpsimd: chains into the gpsimd x-gather with no
                # cross-engine hop, and stays off the congested vector FIFO
                kv = sc.tile([P, 2 * NCORES], f32, tag=f"kv{s}", name=f"kv{s}")
                nc.scalar.dma_start(
                    out=kv[:].rearrange("p (c w) -> p c w", w=2),
                    in_=k_out[t][s][:].rearrange("(c p w) -> p c w", c=NCORES, w=2))
                vals8 = kv[:].rearrange("p (c w) -> p c w", w=2)[:, :, 0]
                idx8 = kv[:].rearrange("p (c w) -> p c w", w=2)[:, :, 1].bitcast(i32)
                gmax = sc.tile([P, 1], f32, tag=f"gm{s}", name=f"gm{s}")
                nc.vector.tensor_reduce(gmax[:], vals8, AxisX, OP.max)
                mask = sc.tile([P, NCORES], u32, tag=f"mk{s}", name=f"mk{s}")
                nc.vector.tensor_tensor(mask[:], vals8,
                                        gmax[:].to_broadcast([P, NCORES]), OP.is_ge)
                cand = sc.tile([P, NCORES], i32, tag=f"cd{s}", name=f"cd{s}")
                nc.vector.memset(cand[:], 0x7FFFFFFF)
                nc.vector.copy_predicated(cand[:], mask[:], idx8)
                tok_new = work.tile([P, 1], i32, tag=f"tok{s}", name=f"tok{s}")
                nc.vector.tensor_reduce(tok_new[:], cand[:], AxisX, OP.min)
                tok_sb[s] = tok_new
                nc.vector.tensor_copy(tokh[s][:, t + 1:t + 2], tok_new[:])

            def ph1(t, s):
                """GRU half: tok(t) -> h'(t) -> coll2 launch.  No vector ops."""
                emit_gh(t, s)
                emit_gather_x(t, s)
                emit_gi(t, s)
                d_pair = emit_gates(t, s)
                emit_hshare(t, s, d_pair)

            bt_c = [None, None]

            def ph2a(t, s):
                """h(t) back -> logits + riding block scans."""
                emit_hback(t, s)
                bt_c[s] = emit_pred(t, s)

            def ph2b(t, s):
                """top-2 select -> exact rescore -> key -> coll1 launch."""
                emit_argmax_rescore(t, s, *bt_c[s])

            # software-pipelined half-phase schedule, emitted in expected
            # execution order so no engine FIFO head-of-line blocks the
            # other stream
            ph1(0, 0)
            for t in range(NSTEPS):
                ph2a(t, 0)
                ph1(t, 1)
                ph2b(t, 0)
                emit_combine(t, 0)
                ph2a(t, 1)
                if t + 1 < NSTEPS:
                    ph1(t + 1, 0)
                ph2b(t, 1)
                emit_combine(t, 1)

            # final: write predictions once per stream
            for s in STREAMS:
                nc.sync.dma_start(out=preds[s * P:(s + 1) * P, :], in_=tokh[s][:])

    return nc


def _prep_inputs(caption, img, embedding, W_cond, b_cond, w_ih, w_hh, b_ih,
                 b_hh, W_pred, b_pred):
    caption = np.asarray(caption).astype(np.int32)
    img = np.ascontiguousarray(np.asarray(img, dtype=np.float32))
    embedding = np.ascontiguousarray(np.asarray(embedding, dtype=np.float32))
    W_pred = np.asarray(W_pred, dtype=np.float32)
    b_pred = np.asarray(b_pred, np.float32)
    b_ih = np.asarray(b_ih, np.float32)
    b_hh = np.asarray(b_hh, np.float32)
    wihT = np.ascontiguousarray(np.asarray(w_ih, np.float32).T)   # [H, 3H]
    whhT = np.ascontiguousarray(np.asarray(w_hh, np.float32).T)
    common = dict(
        emb=embedding,
        wct=np.ascontiguousarray(np.asarray(W_cond, np.float32).T),
        imgT=np.ascontiguousarray(img.T),
        tok0=np.ascontiguousarray(caption[:, 0]),
        ident_in=np.eye(P, dtype=np.float32),
        bcond_row=np.asarray(b_cond, np.float32).reshape(1, H),
    )
    in_maps = []
    for c in range(NCORES):
        base = c * NSHARD
        hi = min(base + NSHARD, VOCAB)
        n_real = max(0, hi - base)
        wpt_c = np.zeros((H, NSHARD), np.float32)
        wpt_c[:, :n_real] = W_pred[base:hi].T
        wrows_c = np.zeros((NSHARD, H + 1), np.float32)
        wrows_c[:n_real, :H] = W_pred[base:hi]
        wrows_c[:n_real, H] = b_pred[base:hi]
        lo, hi_f = c * F, (c + 1) * F
        wih_rc = np.concatenate(
            [wihT[:, lo:hi_f], wihT[:, H + lo:H + hi_f],
             wihT[:, 2 * H + lo:2 * H + hi_f]], axis=1)
        sel = np.zeros((H, F), np.float32)
        sel[lo:hi_f, :] = np.eye(F, dtype=np.float32)
        whh_rc = np.concatenate(
            [whhT[:, lo:hi_f], whhT[:, H + lo:H + hi_f],
             whhT[:, 2 * H + lo:2 * H + hi_f], sel], axis=1)
        bg = b_ih + b_hh
        bgate_row = np.concatenate(
            [bg[lo:hi_f], bg[H + lo:H + hi_f],
             b_ih[2 * H + lo:2 * H + hi_f], b_hh[2 * H + lo:2 * H + hi_f]])
        m = dict(common)
        m["wpt"] = np.ascontiguousarray(wpt_c)
        m["wrows"] = np.ascontiguousarray(wrows_c)
        m["wih_r"] = np.ascontiguousarray(wih_rc)
        m["whh_r"] = np.ascontiguousarray(whh_rc)
        m["base_t"] = np.full((P, 1), base, np.int32)
        m["bgate_row"] = np.ascontiguousarray(bgate_row.reshape(1, 4 * F))
        bp = np.zeros((1, NSHARD), np.float32)
        bp[0, :n_real] = b_pred[base:hi]
        m["bpred_row"] = bp
        in_maps.append(m)
    return in_maps


_CACHED = {}


def kernel(**inputs) -> np.ndarray:
    from concourse.bass_utils import run_bass_kernel_spmd

    in_maps = _prep_inputs(**inputs)
    bcond_nz = bool(np.any(np.asarray(inputs["b_cond"])))
    bgate_nz = bool(np.any(np.asarray(inputs["b_ih"]))
                    or np.any(np.asarray(inputs["b_hh"])))
    bpred_nz = bool(np.any(np.asarray(inputs["b_pred"])))
    key = (bcond_nz, bgate_nz, bpred_nz)
    if key not in _CACHED:
        nc = _build(*key)
        nc.finalize()
        _CACHED[key] = nc
    flags = ("bcond_row", "bgate_row", "bpred_row")
    drop = [f for f, nz in zip(flags, key) if not nz]
    for m in in_maps:
        for f in drop:
            m.pop(f, None)
    res = run_bass_kernel_spmd(_CACHED[key], in_maps, list(range(NCORES)))
    return np.ascontiguousarray(res.results[0]["preds"].astype(np.int32))


if __name__ == "__main__":
    d = np.load("inputs.npz")
    inputs = {k: d[k] for k in d.files}
    out = kernel(**inputs)
    exp = np.load("expected.npy")
    print("match:", np.array_equal(out, exp),
          " mismatches:", int((out != exp).sum()), "/", out.size)
